# revision 1
# baseline (speedup 1.0000x reference)
"""Trainium2 Bass kernel for StyleGAN2-style upsampled Conv1d.

Reference computation (for x:(16,256,4096), weight:(256,256,3), bias:(256,)):
  y = conv_transpose1d(x, weight, stride=2)      # correlation on 2x-dilated x
  z = upfirdn1d(y, [1,3,3,1]/8 * 2)              # depthwise FIR
  out = z + bias                                  # (16, 256, 8192)

The transposed conv + FIR collapse into TWO 3-tap correlations over the
original x grid (even/odd output phases):
  out[:, :, 2j]   = A @x[j-1] + B @x[j]  + C @x[j+1]
  out[:, :, 2j+1] = A'@x[j-1] + B'@x[j]  + C'@x[j+1]
with (w0,w1,w2) = weight taps:
  A  = .75 w0 + .25 w1   B  = .25 w0 + .75 w1 + .75 w2   C  = .25 w2
  A' = .25 w0            B' = .75 w0 + .75 w1 + .25 w2   C' = .25 w1 + .75 w2

On-chip: each 3-tap correlation is 6 accumulating fp32r matmuls (3 taps x
2 K-tiles of 128) into one PSUM bank per 512-position chunk; even/odd
banks are paired so one vector/scalar op per pair drains PSUM, adds bias,
and interleaves the two phases into the final layout.  Sharding:
data-parallel over batch (2 per core x 8 cores).
"""

import numpy as np

import concourse.bass as bass
import concourse.mybir as mybir
import concourse.tile as tile
from concourse import bacc
from concourse.bass_utils import run_bass_kernel_spmd

N, IN_CH, OUT_CH, KERNEL, D = 16, 256, 256, 3, 4096
NCORES = 8
BPC = N // NCORES          # batches per core
DOUT = 2 * D
F32 = mybir.dt.float32
F32R = mybir.dt.float32r

NCHUNK = 512               # matmul moving free dim (= one PSUM bank of fp32)
NCHUNKS = D // NCHUNK      # 8
GROUP = 4                  # psum pairs accumulated concurrently (4 pairs = 8 banks)

_CACHED = {}


def _wblk(phase, tap, k, m):
    return ((phase * 3 + tap) * 2 + k) * 2 + m


def _build_nc(mm_dtype=F32R):
    nc = bacc.Bacc("TRN2", target_bir_lowering=False, debug=False)

    # x arrives host-padded with zero columns at 0 and D+1 (3-tap halo).
    x_t = nc.dram_tensor("x", [BPC, IN_CH, D + 2], F32, kind="ExternalInput")
    # w layout: 24 blocks of (128 K, 128 M); see _wblk
    w_t = nc.dram_tensor("w", [128, 24 * 128], F32, kind="ExternalInput")
    b_t = nc.dram_tensor("b", [128, 2], F32, kind="ExternalInput")
    o_t = nc.dram_tensor("out", [BPC, OUT_CH, DOUT], F32, kind="ExternalOutput")

    with tile.TileContext(nc) as tc:
        with (
            tc.tile_pool(name="wpool", bufs=1) as wpool,
            tc.tile_pool(name="xpool", bufs=2 * BPC) as xpool,
            tc.tile_pool(name="zpool", bufs=6) as zpool,
            tc.tile_pool(name="ppool", bufs=GROUP, space="PSUM") as ppool,
        ):
            w_sb = wpool.tile([128, 24 * 128], mm_dtype)
            nc.gpsimd.dma_start(out=w_sb[:], in_=w_t[:])
            b_sb = wpool.tile([128, 2], F32)
            nc.sync.dma_start(out=b_sb[:], in_=b_t[:])

            # x tiles (128, D+2), cast fp32 -> fp32r by the SWDGE DMAs.
            # Two column-block DMAs per tile; the SWDGE queue is FIFO, so
            # load the first blocks of BOTH K-tiles before any second block
            # (the first matmul group reads both).
            half = GROUP * NCHUNK + 3  # covers chunk group 0 reads
            x_sb = {}
            for bb in range(BPC):
                for k in range(2):
                    x_sb[bb, k] = xpool.tile(
                        [128, D + 2], mm_dtype, tag="x", name=f"x_{bb}_{k}"
                    )
            for bb in range(BPC):
                for blk, (lo, hi) in enumerate([(0, half), (half, D + 2)]):
                    for k in range(2):
                        nc.gpsimd.dma_start(
                            out=x_sb[bb, k][:, lo:hi],
                            in_=x_t[bb, k * 128:(k + 1) * 128, lo:hi],
                        )

            # Pre-warm the PE while inputs load: dummy bf16 matmuls on a
            # memset tile (no DMA dependency -- they start right after the
            # preamble) flip the HAM clock gate to 8/8 and keep it warm
            # until the real work arrives.  The PSUM garbage lands in a
            # pool slot that a real accumulation group's start=True clears.
            warm_bf = wpool.tile([128, 128 + NCHUNK], mybir.dt.bfloat16)
            nc.vector.memset(warm_bf[:], 1.0)
            warm_ps = ppool.tile([128, 2 * NCHUNK], F32, tag="pair", name="warm_ps")
            for _ in range(36):
                nc.tensor.matmul(
                    warm_ps[:, 0:NCHUNK],
                    lhsT=warm_bf[:, 0:128],
                    rhs=warm_bf[:, 128:128 + NCHUNK],
                    start=True,
                    stop=True,
                )

            for bb in range(BPC):
                for m in range(2):
                    bias_ap = b_sb[:, m:m + 1]
                    for g in range(NCHUNKS // GROUP):
                        pairs = [
                            ppool.tile([128, 2 * NCHUNK], F32, tag="pair",
                                       name=f"pair_{bb}_{m}_{g}_{i}")
                            for i in range(GROUP)
                        ]
                        # weight-stationary inner order: each of the 12
                        # (phase,tap,ktile) weights streams GROUP chunks.
                        for phase in range(2):
                            for tap in range(3):
                                for k in range(2):
                                    w_ap = w_sb[:, _wblk(phase, tap, k, m) * 128:][:, :128]
                                    for ci in range(GROUP):
                                        c = g * GROUP + ci
                                        rhs = x_sb[bb, k][:, NCHUNK * c + tap:NCHUNK * c + tap + NCHUNK]
                                        nc.tensor.matmul(
                                            pairs[ci][:, phase * NCHUNK:(phase + 1) * NCHUNK],
                                            lhsT=w_ap,
                                            rhs=rhs,
                                            start=(tap == 0 and k == 0),
                                            stop=(tap == 2 and k == 1),
                                        )
                        for ci in range(GROUP):
                            c = g * GROUP + ci
                            zt = zpool.tile([128, 2 * NCHUNK], F32, tag="z",
                                            name=f"z_{bb}_{m}_{c}")
                            # psum pair is [even(512) | odd(512)]; writing in
                            # (phase, j) order at stride 2 interleaves the two
                            # phases while adding bias -- one op per pair,
                            # pairs alternating between vector and scalar.
                            vout = zt[:].rearrange("p (j two) -> p two j", two=2)
                            vin = pairs[ci][:].rearrange("p (two j) -> p two j", two=2)
                            if ci % 2 == 0:
                                nc.vector.tensor_scalar(
                                    out=vout, in0=vin,
                                    scalar1=bias_ap, scalar2=None,
                                    op0=mybir.AluOpType.add,
                                )
                            else:
                                nc.scalar.activation(
                                    out=vout, in_=vin,
                                    func=mybir.ActivationFunctionType.Identity,
                                    bias=bias_ap,
                                )
                            # Final quadrant's outputs ride the by-then idle
                            # scalar HWDGE queue so the kernel tail is not
                            # serialized behind the sync queue's backlog.
                            oeng = nc.scalar if (bb == 1 and m == 1) else nc.sync
                            oeng.dma_start(
                                out=o_t[bb, m * 128:(m + 1) * 128,
                                        c * 2 * NCHUNK:(c + 1) * 2 * NCHUNK],
                                in_=zt[:],
                            )
    nc.compile()
    return nc


def _host_weights(weight, bias):
    w = np.asarray(weight, dtype=np.float32)
    w0, w1, w2 = w[:, :, 0], w[:, :, 1], w[:, :, 2]
    taps = [
        [0.75 * w0 + 0.25 * w1, 0.25 * w0 + 0.75 * w1 + 0.75 * w2, 0.25 * w2],
        [0.25 * w0, 0.75 * w0 + 0.75 * w1 + 0.25 * w2, 0.25 * w1 + 0.75 * w2],
    ]
    w_host = np.zeros((128, 24 * 128), dtype=np.float32)
    for phase in range(2):
        for tap in range(3):
            for k in range(2):
                for m in range(2):
                    blk = _wblk(phase, tap, k, m)
                    # lhsT block[i, o] = W[phase][tap][m*128+o, k*128+i]
                    wt = taps[phase][tap][m * 128:(m + 1) * 128, k * 128:(k + 1) * 128]
                    w_host[:, blk * 128:(blk + 1) * 128] = wt.T
    b_host = np.asarray(bias, dtype=np.float32).reshape(2, 128).T.copy()
    return w_host, b_host


def _host_x(x):
    x = np.asarray(x, dtype=np.float32)
    return np.ascontiguousarray(np.pad(x, ((0, 0), (0, 0), (1, 1))))


def kernel(x, weight, bias):
    x = _host_x(x)
    w_host, b_host = _host_weights(weight, bias)

    if "nc" not in _CACHED:
        _CACHED["nc"] = _build_nc()
    nc = _CACHED["nc"]

    in_maps = []
    for core in range(NCORES):
        shard = np.ascontiguousarray(x[core * BPC:(core + 1) * BPC])
        in_maps.append({"x": shard, "w": w_host, "b": b_host})

    res = run_bass_kernel_spmd(nc, in_maps, core_ids=list(range(NCORES)))
    out = np.concatenate([np.asarray(r["out"]) for r in res.results], axis=0)
    return out



# revision 3
# speedup vs baseline: 1.0869x; 1.0869x over previous
"""Trainium2 Bass kernel for StyleGAN2-style upsampled Conv1d.

Reference (x:(16,256,4096), w:(256,256,3), b:(256,)):
  y = conv_transpose1d(x, w, stride=2)        # 3 taps on the FINE grid
  z = upfirdn1d(y, [1,3,3,1]/8 * 2)           # depthwise FIR
  out = z + bias                               # (16, 256, 8192)

Key factorization (vs. fusing FIR into the conv weights, which needs 6
channel-mixing taps per coarse position): the transposed conv itself has
only THREE taps per coarse position,
    y_e[i] = w0^T x[i-1] + w2^T x[i]          (even fine phase)
    y_o[i] = w1^T x[i]                        (odd  fine phase)
so matmul work is halved.  The FIR [1,3,3,1] = [1,1]*[1,1]*[1,1] is then
three box-filter levels applied on-chip as fp16 tensor_tensor adds:
    L1: S[i]=y_e[i]+y_o[i]          T[i]=y_o[i]+y_e[i+1]
    L2: P[i]=S[i]+T[i]              Q[i]=T[i]+S[i+1]
    L3: U[i]=P[i]+Q[i]              V[i]=Q[i]+P[i+1]
    out[2i] = 0.25*V[i-1]           out[2i+1] = 0.25*U[i]
The 0.25 is folded into the weights host-side; bias/8 is folded into the
PSUM drain (every y gets +b/8, and the three doubling levels turn it into
+b).  Engine split per core: PE does the 6 accumulating fp16 matmuls per
512-position chunk; the scalar engine drains PSUM->fp16 rows (with bias);
vector+gpsimd split each box pass ~80/20.  The two output phases leave the
device as separate fp16 planes; the host interleaves them and upcasts
(pure layout + dtype, no arithmetic).  Sharding: data-parallel over batch
(2 per core x 8 cores).
"""

import numpy as np

import concourse.bass as bass
import concourse.mybir as mybir
import concourse.tile as tile
from concourse import bacc
from concourse.bass_utils import run_bass_kernel_spmd

N, IN_CH, OUT_CH, KERNEL, D = 16, 256, 256, 3, 4096
NCORES = 8
BPC = N // NCORES          # batches per core
F32 = mybir.dt.float32
F16 = mybir.dt.float16

XW = D + 2                 # x cols: idx -1..4096 at col = idx+1
ROW = 4100                 # one phase row; col = idx+1
NCHUNK = 512
NCHUNKS = D // NCHUNK      # 8
GROUP = 4                  # psum pairs in flight (4 pairs = 8 banks)
SPL = 800                  # gpsimd's column share of each box pass
NWARM = 18

_CACHED = {}


def _wblk(tap, k, m):
    return (tap * 2 + k) * 2 + m


def _build_nc():
    nc = bacc.Bacc("TRN2", target_bir_lowering=False, debug=False)

    x_t = nc.dram_tensor("x", [BPC, IN_CH, XW], F16, kind="ExternalInput")
    w_t = nc.dram_tensor("w", [128, 12 * 128], F16, kind="ExternalInput")
    b_t = nc.dram_tensor("b", [128, 2], F32, kind="ExternalInput")
    # [bb, m, phase, p, j]; phase 0 = V (even fine), 1 = U (odd fine)
    o_t = nc.dram_tensor("out", [BPC, 2, 2, 128, D], F16, kind="ExternalOutput")

    ID = mybir.ActivationFunctionType.Identity
    ADD = mybir.AluOpType.add

    with tile.TileContext(nc) as tc:
        with (
            tc.tile_pool(name="wpool", bufs=1) as wpool,
            tc.tile_pool(name="xpool", bufs=2 * BPC) as xpool,
            tc.tile_pool(name="ypool", bufs=2) as ypool,
            tc.tile_pool(name="stpool", bufs=2) as stpool,
            tc.tile_pool(name="pqpool", bufs=2) as pqpool,
            tc.tile_pool(name="opool", bufs=4) as opool,
            tc.tile_pool(name="ppool", bufs=GROUP, space="PSUM") as ppool,
        ):
            w_sb = wpool.tile([128, 12 * 128], F16)
            nc.sync.dma_start(out=w_sb[:], in_=w_t[:])
            b_sb = wpool.tile([128, 2], F32)
            nc.sync.dma_start(out=b_sb[:], in_=b_t[:])

            # x tiles: [128, XW] fp16; halves on two HWDGE queues so the
            # first matmul group can start early.
            HALF = GROUP * NCHUNK + 8
            x_sb = {}
            for bb in range(BPC):
                for k in range(2):
                    x_sb[bb, k] = xpool.tile([128, XW], F16, tag="x",
                                             name=f"x_{bb}_{k}")
            for bb in range(BPC):
                eng = nc.sync if bb == 0 else nc.scalar
                for k in range(2):
                    for lo, hi in ((0, HALF), (HALF, XW)):
                        eng.dma_start(
                            out=x_sb[bb, k][:, lo:hi],
                            in_=x_t[bb, k * 128:(k + 1) * 128, lo:hi],
                        )

            # PE p-state warm-up on a memset tile (no DMA dependency).
            warm = wpool.tile([128, 128 + NCHUNK], mybir.dt.bfloat16)
            nc.vector.memset(warm[:], 1.0)
            warm_ps = ppool.tile([128, 2 * NCHUNK], F32, tag="pair",
                                 name="warm_ps")
            for _ in range(NWARM):
                nc.tensor.matmul(
                    warm_ps[:, 0:NCHUNK],
                    lhsT=warm[:, 0:128],
                    rhs=warm[:, 128:128 + NCHUNK],
                    start=True,
                    stop=True,
                )

            units = [(bb, m) for bb in range(BPC) for m in range(2)]
            for u, (bb, m) in enumerate(units):
                bias_ap = b_sb[:, m:m + 1]
                # phase rows: ye at col-offset 0 (valid cols 1..4097),
                # yo at offset ROW (valid cols 0..4097); col = idx+1
                yrows = ypool.tile([128, 2 * ROW], F16, tag="y",
                                   name=f"y_{u}")
                yv3 = yrows[:].rearrange("p (h w) -> p h w", h=2)

                for g in range(NCHUNKS // GROUP):
                    pairs = [
                        ppool.tile([128, 2 * NCHUNK], F32, tag="pair",
                                   name=f"pair_{u}_{g}_{i}")
                        for i in range(GROUP)
                    ]
                    # weight-stationary: each (tap,k) streams GROUP chunks.
                    # ye half: taps 0,2 (start on first, stop on last);
                    # yo half: tap 1.  rhs col base: tap0 -> p0 (x[i-1]),
                    # taps 1,2 -> p0+1 (x[i]).
                    sched = [(0, 0, 0, True, False), (0, 0, 1, False, False),
                             (2, 1, 0, False, False), (2, 1, 1, False, True),
                             (1, 1, 0, True, False), (1, 1, 1, False, True)]
                    for tap, coff, k, st, sp in sched:
                        w_ap = w_sb[:, _wblk(tap, k, m) * 128:][:, :128]
                        half = 0 if tap != 1 else NCHUNK
                        for ci in range(GROUP):
                            p0 = (g * GROUP + ci) * NCHUNK
                            nc.tensor.matmul(
                                pairs[ci][:, half:half + NCHUNK],
                                lhsT=w_ap,
                                rhs=x_sb[bb, k][:, p0 + coff:p0 + coff + NCHUNK],
                                start=st,
                                stop=sp,
                            )
                    for ci in range(GROUP):
                        c0 = (g * GROUP + ci) * NCHUNK
                        nc.scalar.activation(
                            out=yv3[:, :, c0 + 1:c0 + 513],
                            in_=pairs[ci][:].rearrange("p (h w) -> p h w", h=2),
                            func=ID,
                            bias=bias_ap,
                        )

                # border columns via tiny matmuls on the zero-padded x:
                # mini[0] = ye[4096], mini[1] = yo[-1](=0), mini[2] = yo[4096](=0)
                mini = ppool.tile([128, 2 * NCHUNK], F32, tag="pair",
                                  name=f"mini_{u}")
                for k in range(2):
                    nc.tensor.matmul(
                        mini[:, 0:1], lhsT=w_sb[:, _wblk(0, k, m) * 128:][:, :128],
                        rhs=x_sb[bb, k][:, D:D + 1], start=(k == 0), stop=False)
                    nc.tensor.matmul(
                        mini[:, 0:1], lhsT=w_sb[:, _wblk(2, k, m) * 128:][:, :128],
                        rhs=x_sb[bb, k][:, D + 1:D + 2], start=False, stop=(k == 1))
                for k in range(2):
                    nc.tensor.matmul(
                        mini[:, 1:3], lhsT=w_sb[:, _wblk(1, k, m) * 128:][:, :128],
                        rhs=x_sb[bb, k][:, 0:XW:D + 1], start=(k == 0), stop=(k == 1))
                nc.scalar.activation(out=yrows[:, 4097:4098], in_=mini[:, 0:1],
                                     func=ID, bias=bias_ap)
                nc.scalar.activation(out=yrows[:, ROW:2 * ROW:4097],
                                     in_=mini[:, 1:3], func=ID, bias=bias_ap)

                # box cascade, fp16; each pass split gpsimd/vector at SPL
                st_rows = stpool.tile([128, 2 * ROW], F16, tag="st",
                                      name=f"st_{u}")
                pq_rows = pqpool.tile([128, 2 * ROW], F16, tag="pq",
                                      name=f"pq_{u}")
                u_out = opool.tile([128, D], F16, tag="o", name=f"u_{u}")
                v_out = opool.tile([128, D], F16, tag="o", name=f"v_{u}")
                YE, YO = 0, ROW
                S, T = 0, ROW
                P, Q = 0, ROW
                passes = [
                    # (out tile, out base, in0 tile, in0 base, in1 tile, in1 base, len)
                    (st_rows, S + 1, yrows, YE + 1, yrows, YO + 1, 4097),
                    (st_rows, T + 0, yrows, YO + 0, yrows, YE + 1, 4097),
                    (pq_rows, P + 1, st_rows, S + 1, st_rows, T + 1, 4096),
                    (pq_rows, Q + 0, st_rows, T + 0, st_rows, S + 1, 4097),
                    (u_out, 0, pq_rows, P + 1, pq_rows, Q + 1, 4096),
                    (v_out, 0, pq_rows, Q + 0, pq_rows, P + 1, 4096),
                ]
                for ot, ob, t0, b0, t1, b1, ln in passes:
                    nc.gpsimd.tensor_tensor(
                        ot[:, ob:ob + SPL],
                        t0[:, b0:b0 + SPL], t1[:, b1:b1 + SPL], ADD)
                    nc.vector.tensor_tensor(
                        ot[:, ob + SPL:ob + ln],
                        t0[:, b0 + SPL:b0 + ln], t1[:, b1 + SPL:b1 + ln], ADD)

                oeng = nc.scalar if u == len(units) - 1 else nc.sync
                oeng.dma_start(out=o_t[bb, m, 0], in_=v_out[:])
                oeng.dma_start(out=o_t[bb, m, 1], in_=u_out[:])
    nc.compile()
    return nc


def _host_weights(weight, bias):
    w = np.asarray(weight, dtype=np.float32) * 0.25
    w_host = np.zeros((128, 12 * 128), dtype=np.float16)
    for tap in range(3):
        for k in range(2):
            for m in range(2):
                blk = _wblk(tap, k, m)
                wt = w[m * 128:(m + 1) * 128, k * 128:(k + 1) * 128, tap]
                w_host[:, blk * 128:(blk + 1) * 128] = wt.T.astype(np.float16)
    b_host = (np.asarray(bias, dtype=np.float32) / 8.0).reshape(2, 128).T.copy()
    return w_host, b_host


def _host_x(x):
    x = np.asarray(x, dtype=np.float32)
    xp = np.zeros((N, IN_CH, XW), dtype=np.float16)
    xp[:, :, 1:D + 1] = x.astype(np.float16)
    return xp


def _assemble(res_list):
    # device output: [BPC, 2(m), 2(phase), 128, D] fp16 per core.
    # Interleave the two fine phases and upcast (layout + dtype only).
    out = np.empty((N, OUT_CH, 2 * D), dtype=np.float32)
    for core, r in enumerate(res_list):
        o = np.asarray(r["out"])
        for bb in range(BPC):
            n = core * BPC + bb
            for m in range(2):
                ch = slice(m * 128, (m + 1) * 128)
                out[n, ch, 0::2] = o[bb, m, 0].astype(np.float32)
                out[n, ch, 1::2] = o[bb, m, 1].astype(np.float32)
    return out


def kernel(x, weight, bias):
    xh = _host_x(x)
    w_host, b_host = _host_weights(weight, bias)

    if "nc" not in _CACHED:
        _CACHED["nc"] = _build_nc()
    nc = _CACHED["nc"]

    in_maps = []
    for core in range(NCORES):
        shard = np.ascontiguousarray(xh[core * BPC:(core + 1) * BPC])
        in_maps.append({"x": shard, "w": w_host, "b": b_host})

    res = run_bass_kernel_spmd(nc, in_maps, core_ids=list(range(NCORES)))
    return _assemble(res.results)


# revision 8
# speedup vs baseline: 1.2292x; 1.1310x over previous
"""Trainium2 Bass kernel for StyleGAN2-style upsampled Conv1d.

Reference (x:(16,256,4096), w:(256,256,3), b:(256,)):
  y = conv_transpose1d(x, w, stride=2)        # 3 taps on the FINE grid
  z = upfirdn1d(y, [1,3,3,1]/8 * 2)           # depthwise FIR
  out = z + bias                               # (16, 256, 8192)

The transposed conv has only THREE channel-mixing taps per coarse
position (vs 6 when the FIR is folded in):
    y_e[i] = w0^T x[i-1] + w2^T x[i]           (even fine phase)
    y_o[i] = w1^T x[i]                         (odd  fine phase)
The FIR [1,3,3,1] = [1,1]*[1,1]*[1,1] is a 3-level box cascade in the
two-phase domain:
    L1: S[i]=y_e[i]+y_o[i]          T[i]=y_o[i]+y_e[i+1]
    L2: P[i]=S[i]+T[i]              Q[i]=T[i]+S[i+1]
    L3: U[i]=P[i]+Q[i]              V[i]=Q[i]+P[i+1]
    out[2i] = 0.25*V[i-1]           out[2i+1] = 0.25*U[i]
(0.25 folded into the weights host-side.)

L1 can equivalently run on the PE -- S,T are themselves 2-tap convs:
    S[i] = .25*w0^T x[i-1] + .25*(w1+w2)^T x[i]
    T[i] = .25*(w0+w1)^T x[i] + .25*w2^T x[i+1]
Total work is invariant (each box level is 32.7k PSUM rows on PE or
32.7k fp16 adds on DVE), so chunks are split between the two forms to
balance the engines: per 512-position chunk, the first C4 chunks of
each row compute S,T directly (8 matmuls), the rest compute y_e,y_o
(6 matmuls) plus an L1 pass on DVE.  The scalar engine drains PSUM ->
fp16 rows with the bias folded in (bias/4 on S,T drains, bias/8 on
y drains; the cascade doublings turn both into +bias).  The two output
phases leave as separate fp16 planes; the host interleaves and upcasts
(layout + dtype only).  Sharding: data-parallel over batch (2/core).
"""

import numpy as np

import concourse.bass as bass
import concourse.mybir as mybir
import concourse.tile as tile
from concourse import bacc
from concourse.bass_utils import run_bass_kernel_spmd

N, IN_CH, OUT_CH, KERNEL, D = 16, 256, 256, 3, 4096
NCORES = 8
BPC = N // NCORES          # batches per core
F32 = mybir.dt.float32
F16 = mybir.dt.float16

XW = D + 2                 # x cols: idx -1..4096 at col = idx+1
ROW = 4100                 # phase-row width; col = idx+1
NCHUNK = 512
NCHUNKS = D // NCHUNK      # 8
GROUP = 4                  # psum pairs in flight (4 pairs = 8 banks)
C4 = 4                     # chunks per unit computed in S,T (4-tap) form
NWARM = 18

# weight-block ids: A=.25w0 B=.25w1 C=.25w2 D4=.25(w1+w2) E=.25(w0+w1)
WA, WB, WC, WD, WE = 0, 1, 2, 3, 4

_CACHED = {}


def _wblk(t, k, m):
    return (t * 2 + k) * 2 + m


def _build_nc():
    nc = bacc.Bacc("TRN2", target_bir_lowering=False, debug=False)

    x_t = nc.dram_tensor("x", [BPC, IN_CH, XW], F16, kind="ExternalInput")
    w_t = nc.dram_tensor("w", [128, 20 * 128], F16, kind="ExternalInput")
    b_t = nc.dram_tensor("b", [128, 4], F32, kind="ExternalInput")
    # [bb, m, phase, p, j]; phase 0 = V (even fine), 1 = U (odd fine)
    o_t = nc.dram_tensor("out", [BPC, 2, 2, 128, D], F16, kind="ExternalOutput")

    ID = mybir.ActivationFunctionType.Identity
    ADD = mybir.AluOpType.add
    c4c = C4 * NCHUNK

    with tile.TileContext(nc) as tc:
        with (
            tc.tile_pool(name="wpool", bufs=1) as wpool,
            tc.tile_pool(name="xpool", bufs=2 * BPC) as xpool,
            tc.tile_pool(name="ypool", bufs=2) as ypool,
            tc.tile_pool(name="stpool", bufs=2) as stpool,
            tc.tile_pool(name="pqpool", bufs=2) as pqpool,
            tc.tile_pool(name="opool", bufs=4) as opool,
            tc.tile_pool(name="ppool", bufs=GROUP, space="PSUM") as ppool,
        ):
            w_sb = wpool.tile([128, 20 * 128], F16)
            nc.sync.dma_start(out=w_sb[:], in_=w_t[:])
            b_sb = wpool.tile([128, 4], F32)
            nc.sync.dma_start(out=b_sb[:], in_=b_t[:])
            scr = wpool.tile([128, 1], F16)

            # x tiles: [128, XW] fp16, halves so early matmuls start early.
            HALF = GROUP * NCHUNK + 8
            x_sb = {}
            for bb in range(BPC):
                for k in range(2):
                    x_sb[bb, k] = xpool.tile([128, XW], F16, tag="x",
                                             name=f"x_{bb}_{k}")
            for bb in range(BPC):
                eng = nc.sync if bb == 0 else nc.gpsimd
                for k in range(2):
                    # the 2 right-edge pad cols first: border minis need them
                    for lo, hi in ((D, XW), (0, HALF), (HALF, D)):
                        eng.dma_start(
                            out=x_sb[bb, k][:, lo:hi],
                            in_=x_t[bb, k * 128:(k + 1) * 128, lo:hi],
                        )

            # activation-table preload (one tiny act) + PE p-state warm-up
            nc.scalar.activation(out=scr[:], in_=b_sb[:, 0:1], func=ID,
                                 bias=b_sb[:, 0:1])
            warm = wpool.tile([128, 128 + NCHUNK], mybir.dt.bfloat16)
            nc.vector.memset(warm[:], 1.0)
            warm_ps = ppool.tile([128, 2 * NCHUNK], F32, tag="pair",
                                 name="warm_ps")
            for _ in range(NWARM):
                nc.tensor.matmul(
                    warm_ps[:, 0:NCHUNK],
                    lhsT=warm[:, 0:128],
                    rhs=warm[:, 128:128 + NCHUNK],
                    start=True,
                    stop=True,
                )

            def mm(pair, half, t, k, m, bb, cols, st, sp):
                nc.tensor.matmul(
                    pair[:, half:half + cols.stop - cols.start],
                    lhsT=w_sb[:, _wblk(t, k, m) * 128:][:, :128],
                    rhs=x_sb[bb, k][:, cols],
                    start=st, stop=sp,
                )

            units = [(bb, m) for bb in range(BPC) for m in range(2)]
            last = len(units) - 1
            for u, (bb, m) in enumerate(units):
                b8 = b_sb[:, m:m + 1]        # bias/8 (y drains)
                b4 = b_sb[:, 2 + m:3 + m]    # bias/4 (S,T drains)
                st_rows = stpool.tile([128, 2 * ROW], F16, tag="st",
                                      name=f"st_{u}")
                st3 = st_rows[:].rearrange("p (h w) -> p h w", h=2)
                if C4 < 8:
                    yrows = ypool.tile([128, 2 * ROW], F16, tag="y",
                                       name=f"y_{u}")
                    yv3 = yrows[:].rearrange("p (h w) -> p h w", h=2)

                # border minis (tiny matmuls on the zero-padded x edges)
                mini = ppool.tile([128, 2 * NCHUNK], F32, tag="pair",
                                  name=f"mini_{u}")
                nmini = 0
                drains = []
                if C4 > 0:  # T[-1] = E@x[-1] + C@x[0]
                    for k in range(2):
                        mm(mini, nmini, WE, k, m, bb, slice(0, 1), k == 0, False)
                        mm(mini, nmini, WC, k, m, bb, slice(1, 2), False, k == 1)
                    drains.append((st_rows[:, ROW:ROW + 1], nmini, b4))
                    nmini += 1
                else:       # yo[-1] = B@x[-1] (zero)
                    for k in range(2):
                        mm(mini, nmini, WB, k, m, bb, slice(0, 1), k == 0, k == 1)
                    drains.append((yrows[:, ROW:ROW + 1], nmini, b8))
                    nmini += 1
                if C4 < 8:  # ye[4096] = A@x[4095]+C@x[4096]; yo[4096] = B@x[4096]
                    for k in range(2):
                        mm(mini, nmini, WA, k, m, bb, slice(D, D + 1), k == 0, False)
                        mm(mini, nmini, WC, k, m, bb, slice(D + 1, D + 2), False, k == 1)
                    drains.append((yrows[:, D + 1:D + 2], nmini, b8))
                    nmini += 1
                    for k in range(2):
                        mm(mini, nmini, WB, k, m, bb, slice(D + 1, D + 2), k == 0, k == 1)
                    drains.append((yrows[:, ROW + D + 1:ROW + D + 2], nmini, b8))
                    nmini += 1
                else:       # S[4096] = A@x[4095] + D4@x[4096]
                    for k in range(2):
                        mm(mini, nmini, WA, k, m, bb, slice(D, D + 1), k == 0, False)
                        mm(mini, nmini, WD, k, m, bb, slice(D + 1, D + 2), False, k == 1)
                    drains.append((st_rows[:, D + 1:D + 2], nmini, b4))
                    nmini += 1
                for out_ap, col, bias_ap in drains:
                    nc.scalar.activation(out=out_ap, in_=mini[:, col:col + 1],
                                         func=ID, bias=bias_ap)

                for g in range(NCHUNKS // GROUP):
                    chunks = [g * GROUP + ci for ci in range(GROUP)]
                    pairs = {}
                    for c in chunks:
                        pairs[c] = ppool.tile([128, 2 * NCHUNK], F32,
                                              tag="pair", name=f"pair_{u}_{c}")
                    # weight-stationary over the group, per tap-mode
                    sched4 = [(WA, 0, 0, 0, True, False), (WA, 0, 1, 0, False, False),
                              (WD, 1, 0, 0, False, False), (WD, 1, 1, 0, False, True),
                              (WE, 1, 0, 1, True, False), (WE, 1, 1, 1, False, False),
                              (WC, 2, 0, 1, False, False), (WC, 2, 1, 1, False, True)]
                    sched3 = [(WA, 0, 0, 0, True, False), (WA, 0, 1, 0, False, False),
                              (WC, 1, 0, 0, False, False), (WC, 1, 1, 0, False, True),
                              (WB, 1, 0, 1, True, False), (WB, 1, 1, 1, False, True)]
                    for mode, sched in ((4, sched4), (3, sched3)):
                        sel = [c for c in chunks if (c < C4) == (mode == 4)]
                        if not sel:
                            continue
                        for t, coff, k, half, st, sp in sched:
                            for c in sel:
                                p0 = c * NCHUNK
                                mm(pairs[c], half * NCHUNK, t, k, m, bb,
                                   slice(p0 + coff, p0 + coff + NCHUNK), st, sp)
                    for c in chunks:
                        c0 = c * NCHUNK
                        tgt = st3 if c < C4 else yv3
                        nc.scalar.activation(
                            out=tgt[:, :, c0 + 1:c0 + 513],
                            in_=pairs[c][:].rearrange("p (h w) -> p h w", h=2),
                            func=ID,
                            bias=b4 if c < C4 else b8,
                        )

                # cascade on DVE (fp16 tensor_tensor); L1 only on 3-tap cols
                pq_rows = pqpool.tile([128, 2 * ROW], F16, tag="pq",
                                      name=f"pq_{u}")
                u_out = opool.tile([128, D], F16, tag="o", name=f"u_{u}")
                v_out = opool.tile([128, D], F16, tag="o", name=f"v_{u}")
                S, T = 0, ROW
                P, Q = 0, ROW
                passes = []
                if C4 < 8:
                    tlo = c4c if C4 > 0 else -1
                    passes += [
                        (st_rows, S + c4c + 1, yrows, c4c + 1, yrows, ROW + c4c + 1,
                         4097 - c4c),
                        (st_rows, T + tlo + 1, yrows, ROW + tlo + 1, yrows, tlo + 2,
                         4096 - tlo),
                    ]
                passes += [
                    (pq_rows, P + 1, st_rows, S + 1, st_rows, T + 1, 4096),
                    (pq_rows, Q + 0, st_rows, T + 0, st_rows, S + 1, 4097),
                    (u_out, 0, pq_rows, P + 1, pq_rows, Q + 1, 4096),
                    (v_out, 0, pq_rows, Q + 0, pq_rows, P + 1, 4096),
                ]
                if u == last:
                    # split passes so the second halves form a short tail
                    cuts = [2053, 2053, 2053, 2052, 2051, 2051]
                    if C4 == 8:
                        cuts = cuts[2:]
                    for (ot, ob, t0, b0, t1, b1, ln), cut in zip(passes, cuts):
                        h = max(0, min(cut - (ob % ROW), ln))
                        if h > 0:
                            nc.vector.tensor_tensor(
                                ot[:, ob:ob + h], t0[:, b0:b0 + h],
                                t1[:, b1:b1 + h], ADD)
                        if h < ln:
                            nc.vector.tensor_tensor(
                                ot[:, ob + h:ob + ln], t0[:, b0 + h:b0 + ln],
                                t1[:, b1 + h:b1 + ln], ADD)
                else:
                    for ot, ob, t0, b0, t1, b1, ln in passes:
                        nc.vector.tensor_tensor(
                            ot[:, ob:ob + ln], t0[:, b0:b0 + ln],
                            t1[:, b1:b1 + ln], ADD)

                if u == last:
                    HB = 2051
                    nc.sync.dma_start(out=o_t[bb, m, 0, :, 0:HB],
                                      in_=v_out[:, 0:HB])
                    nc.sync.dma_start(out=o_t[bb, m, 1, :, 0:HB],
                                      in_=u_out[:, 0:HB])
                    nc.scalar.dma_start(out=o_t[bb, m, 0, :, HB:D],
                                        in_=v_out[:, HB:D])
                    nc.scalar.dma_start(out=o_t[bb, m, 1, :, HB:D],
                                        in_=u_out[:, HB:D])
                else:
                    nc.sync.dma_start(out=o_t[bb, m, 0], in_=v_out[:])
                    nc.sync.dma_start(out=o_t[bb, m, 1], in_=u_out[:])
    nc.compile()
    return nc


def _host_weights(weight, bias):
    w = np.asarray(weight, dtype=np.float32) * 0.25
    w0, w1, w2 = w[:, :, 0], w[:, :, 1], w[:, :, 2]
    taps = [w0, w1, w2, w1 + w2, w0 + w1]
    w_host = np.zeros((128, 20 * 128), dtype=np.float16)
    for t in range(5):
        for k in range(2):
            for m in range(2):
                blk = _wblk(t, k, m)
                wt = taps[t][m * 128:(m + 1) * 128, k * 128:(k + 1) * 128]
                w_host[:, blk * 128:(blk + 1) * 128] = wt.T.astype(np.float16)
    b = np.asarray(bias, dtype=np.float32)
    b_host = np.stack([b[0:128] / 8, b[128:256] / 8,
                       b[0:128] / 4, b[128:256] / 4], axis=1).copy()
    return w_host, b_host


def _host_x(x):
    x = np.asarray(x, dtype=np.float32)
    xp = np.zeros((N, IN_CH, XW), dtype=np.float16)
    xp[:, :, 1:D + 1] = x.astype(np.float16)
    return xp


def _assemble(res_list):
    # device output: [BPC, 2(m), 2(phase), 128, D] fp16 per core.
    # Interleave the two fine phases and upcast (layout + dtype only).
    out = np.empty((N, OUT_CH, 2 * D), dtype=np.float32)
    for core, r in enumerate(res_list):
        o = np.asarray(r["out"])
        for bb in range(BPC):
            n = core * BPC + bb
            for m in range(2):
                ch = slice(m * 128, (m + 1) * 128)
                out[n, ch, 0::2] = o[bb, m, 0].astype(np.float32)
                out[n, ch, 1::2] = o[bb, m, 1].astype(np.float32)
    return out


def kernel(x, weight, bias):
    xh = _host_x(x)
    w_host, b_host = _host_weights(weight, bias)

    if "nc" not in _CACHED:
        _CACHED["nc"] = _build_nc()
    nc = _CACHED["nc"]

    in_maps = []
    for core in range(NCORES):
        shard = np.ascontiguousarray(xh[core * BPC:(core + 1) * BPC])
        in_maps.append({"x": shard, "w": w_host, "b": b_host})

    res = run_bass_kernel_spmd(nc, in_maps, core_ids=list(range(NCORES)))
    return _assemble(res.results)
